# revision 9
# baseline (speedup 1.0000x reference)
"""Bass/Tile kernel for nn_Attention_9234179687166 on 8 TRN2 NeuronCores.

Reference computation per batch b (B=32, L=K=D=1024):
    q      = query @ W_in.T                    # [L, D]
    scores = q @ context.T                     # [L, K]
    w      = masked_softmax(scores, mask)      # multiplicative mask + renorm
    mix    = w @ context                       # [L, D]
    out    = tanh(concat([mix, q]) @ W_out.T)  # [L, D]

Sharding: data-parallel over batch, 4 batches per core, weights replicated.

The PE runs ONLY 512-wide f16 matmuls (5*1024^3 MACs/batch = the roofline);
every per-batch transpose is done by the DMA xbar (SBUF->SBUF f16
`dma_start(..., transpose=True)`, ~1.6us per [128,1024] tile, serialized on
the DMA complex but fully off the PE). Transposed operands live in "grouped"
layouts ([P, blk, sub, P]) because the xbar needs contiguous destinations;
matmul moving operands use 3D APs ([128, 4 x stride, 128 contig] = 512 free)
to read them. W_in/W_out setup transposes stay on the PE (f32, overlapped
with the initial weight DMAs when the PE is idle anyway).

Masked softmax (mask m in {0,1}, scores s):
    u = (s + 4096)*m  (masked -> 0), e = exp(u - max(u)) has masked lanes
    exp(-~4096) == 0, and w = e/sum(e) matches the reference up to ~1e-10.
    The normalization rec = 1/sum(e) is folded into e BEFORE the w
    transpose (DVE tensor_scalar with per-partition rec), so step 5 is a
    single PSUM accumulation over [mixT; qTr] @ W_outT with tanh straight
    out of PSUM.

Per-half software pipeline (PE order):
    scores(h) lj=0..3 -> step1(h+1) (covers the softmax/transpose tail)
    -> step4(h) -> step5(h).
"""

import sys

sys.path.insert(0, "/opt/trn_rl_repo")

import numpy as np

P = 128
D = 1024
TWO_D = 2048
DT = D // P      # 8 tiles over D
CT = TWO_D // P  # 16 tiles over 2D
LARGE = 4096.0
N_CORES = 8
B_FULL = 32
NB = B_FULL // N_CORES  # batches per core

_prog_cache = {}
last_results = None  # BassKernelResults of the most recent kernel() call


def build_program(nb, L, K=1024, reps=1):
    import concourse.mybir as mybir
    import concourse.tile as tile
    from concourse import bacc
    from concourse.masks import make_identity

    f32 = mybir.dt.float32
    f32r = mybir.dt.float32r
    f16 = mybir.dt.float16
    i32 = mybir.dt.int32
    Alu = mybir.AluOpType
    Act = mybir.ActivationFunctionType
    KT = K // P
    LH = min(512, L)      # l-half width (free dim of step1/4/5 matmuls)
    NHALF = L // LH
    LJ = LH // P          # 128-row l tiles per half
    KH = K // 512         # 512-wide k chunks for the scores matmul

    nc = bacc.Bacc("TRN2", target_bir_lowering=False, debug=False,
                   num_devices=N_CORES)
    q_d = nc.dram_tensor("query", [nb, L, D], f32, kind="ExternalInput")
    c_d = nc.dram_tensor("context", [nb, K, D], f32, kind="ExternalInput")
    m_d = nc.dram_tensor("mask", [nb, L, K], i32, kind="ExternalInput")
    win_d = nc.dram_tensor("W_in", [D, D], f32, kind="ExternalInput")
    wout_d = nc.dram_tensor("W_out", [D, TWO_D], f32, kind="ExternalInput")
    out_d = nc.dram_tensor("out", [nb, L, D], f32, kind="ExternalOutput")

    copy_flip = [0]

    def grouped_copy(nc, dst_ap, src_ap):
        # Alternate psum->sbuf copies between DVE and ACT to halve the
        # per-engine copy latency chain.
        if copy_flip[0] % 2 == 0:
            nc.vector.tensor_copy(dst_ap, src_ap)
        else:
            nc.scalar.activation(dst_ap, src_ap, mybir.ActivationFunctionType.Copy)
        copy_flip[0] += 1

    with tile.TileContext(nc) as tc:
        with (
            tc.tile_pool(name="const", bufs=1) as constp,
            tc.tile_pool(name="wres", bufs=1) as wres,
            tc.tile_pool(name="ps_big", bufs=3, space="PSUM") as ps_big,
            tc.tile_pool(name="ps_mm", bufs=2, space="PSUM") as ps_mm,
        ):
            ident = constp.tile([P, P], f32)
            make_identity(nc, ident)
            ident_bf = constp.tile([P, P], f16)
            nc.vector.tensor_copy(ident_bf[:], ident[:])

            W_inT = wres.tile([P, DT, D], f16)       # [d_in, d_out_blk, e]
            W_outT = wres.tile([P, CT, D], f16)      # [c_in, c_blk, d_out]

            def transpose_pack4(nc, dst_tile, dst_t0, dst_col0, src_ap_fn, n,
                                idn, dtype):
                """n PE transposes (groups of up to 4) of 128x128 slices
                (setup-only; per-batch transposes use the DMA xbar)."""
                g = 0
                while g < n:
                    gn = min(4, n - g)
                    # tag "mm" slots are 2 KB/partition: [P, 4P] f32 or
                    # [P, 8P] f16 (only the first 4P columns are used).
                    cols = 4 * P if dtype == f32 else 8 * P
                    tp = ps_mm.tile([P, cols], dtype, tag="mm")
                    for i in range(gn):
                        src = src_ap_fn(g + i)
                        if dtype == f32r and src.dtype == f32:
                            src = src.bitcast(f32r)
                        nc.tensor.transpose(
                            tp[:, i * P:(i + 1) * P], src, idn[:])
                    grouped_copy(
                        nc,
                        dst_tile[:, dst_t0 + g:dst_t0 + g + gn,
                                 dst_col0:dst_col0 + P],
                        tp[:, :gn * P],
                    )
                    g += gn

            with (
                tc.tile_pool(name="ctx", bufs=1) as ctxp,
                tc.tile_pool(name="acts", bufs=1) as actsp,
                tc.tile_pool(name="rot", bufs=4) as natp,
                tc.tile_pool(name="sm", bufs=2) as smp,
            ):
                ctx_tiles = {}

                def emit_ctx_stage(b):
                    # context: SWDGE cast-load (f32 DRAM -> f16 SBUF) then
                    # xbar-transpose to grouped ctxTg[d_lo, ki, di, k_lo]
                    # (scores moving operand). Loads ride the gpsimd queue,
                    # transposes the sync queue, so neither blocks the other.
                    ctx_bf = ctxp.tile([P, KT, D], f16, tag="ctxbf")
                    ctxTg = ctxp.tile([P, KT, DT, P], f16, tag="ctxTg")
                    for ki in range(KT):
                        nc.gpsimd.dma_start(ctx_bf[:, ki, :],
                                            c_d[b, ki * P:(ki + 1) * P, :])
                        nc.sync.dma_start(ctxTg[:, ki, :, :], ctx_bf[:, ki, :],
                                          transpose=True)
                    ctx_tiles[b] = (ctxTg, ctx_bf)

                def emit_query_stage(b, h):
                    # query: SWDGE cast-load then xbar-transpose each l-tile
                    # to grouped qTg[d_lo, lj, di, l_lo] (step1 moving).
                    l0 = h * LH
                    qTg = actsp.tile([P, LJ, DT, P], f16, tag="qTg", bufs=2)
                    for lj in range(LJ):
                        qh = smp.tile([P, D], f16, tag="qh", bufs=4)
                        nc.gpsimd.dma_start(
                            qh[:], q_d[b, l0 + lj * P: l0 + (lj + 1) * P, :])
                        nc.sync.dma_start(qTg[:, lj, :, :], qh[:],
                                          transpose=True)
                    return qTg

                def emit_step1(qTg):
                    # qTr[e, l] = W_inT.T @ qTg (f16), e on partitions.
                    qTr = actsp.tile([P, DT, LH], f16, tag="qTr", bufs=2)
                    for ei in range(DT):
                        psq = ps_mm.tile([P, LH], f32, tag="mm")
                        for di in range(DT):
                            nc.tensor.matmul(
                                psq[:],
                                W_inT[:, di, ei * P:(ei + 1) * P],
                                qTg[:, :, di, :],
                                start=(di == 0), stop=(di == DT - 1),
                            )
                        grouped_copy(nc, qTr[:, ei, :], psq[:])
                    return qTr

                def emit_half(b, h, qTr, next_bh):
                    l0 = h * LH
                    ctxTg, ctx_bf = ctx_tiles[b]

                    # ---- step 2 + masked softmax per l-tile ----
                    wTg = actsp.tile([P, LJ, KT, P], f16, tag="wTg")
                    qTg_next = None
                    for lj in range(LJ):
                        mi = smp.tile([P, K], i32, tag="mask", bufs=2)
                        nc.gpsimd.dma_start(
                            mi[:], m_d[b, l0 + lj * P: l0 + (lj + 1) * P, :])
                        pss = ps_big.tile([P, K], f32, tag="big")
                        for ei in range(DT):
                            for kh in range(KH):
                                nc.tensor.matmul(
                                    pss[:, kh * 512:(kh + 1) * 512],
                                    qTr[:, ei, lj * P:(lj + 1) * P],
                                    ctxTg[:, kh * 4:(kh + 1) * 4, ei, :],
                                    start=(ei == 0), stop=(ei == DT - 1),
                                )
                        st = smp.tile([P, 4], f32, tag="stats", bufs=2)
                        # u = (s + LARGE) * m in SBUF (frees the scores PSUM
                        # right away).
                        u_t = smp.tile([P, K], f32, tag="u", bufs=2)
                        nc.vector.scalar_tensor_tensor(
                            u_t[:], pss[:], LARGE, mi[:],
                            op0=Alu.add, op1=Alu.mult)
                        nc.vector.tensor_reduce(
                            st[:, 0:1], u_t[:], axis=mybir.AxisListType.X,
                            op=Alu.max, negate=True)
                        e_sb = smp.tile([P, K], f16, tag="e", bufs=2)
                        nc.scalar.activation(
                            e_sb[:], u_t[:], Act.Exp,
                            bias=st[:, 0:1], accum_out=st[:, 1:2])
                        nc.vector.reciprocal(st[:, 2:3], st[:, 1:2])
                        # w = e * rec (pre-normalized so step 5 is a single
                        # accumulation), then xbar-transpose to grouped wTg.
                        # The scale runs on ACT right behind its exp.
                        w_sb = smp.tile([P, K], f16, tag="w", bufs=2)
                        nc.scalar.activation(
                            w_sb[:], e_sb[:], Act.Copy, scale=st[:, 2:3])
                        nc.sync.dma_start(wTg[:, lj, :, :], w_sb[:],
                                          transpose=True)
                        if lj == 0:
                            # Prefetch the next half's query pipeline now so
                            # its DMAs/transposes run during this half.
                            if next_bh is not None:
                                qTg_next = emit_query_stage(*next_bh)

                    # ---- step 1 for the NEXT half: fills the PE while this
                    # half's softmax/transpose tail drains. ----
                    qTr_next = emit_step1(qTg_next) if qTg_next is not None else None

                    # ---- step 4: mixT[d', l] = ctx_bf.T @ wTg (f16) ----
                    mixT = actsp.tile([P, DT, LH], f16, tag="mixT", bufs=2)
                    for di in range(DT):
                        psm = ps_mm.tile([P, LH], f32, tag="mm")
                        for ki in range(KT):
                            nc.tensor.matmul(
                                psm[:],
                                ctx_bf[:, ki, di * P:(di + 1) * P],
                                wTg[:, :, ki, :],
                                start=(ki == 0), stop=(ki == KT - 1),
                            )
                        grouped_copy(nc, mixT[:, di, :], psm[:])

                    # ---- step 5: out = tanh([mixT; qTr] @ W_outT), single
                    # PSUM accumulation (w pre-normalized). ----
                    for lj in range(LJ):
                        pso = ps_big.tile([P, K], f32, tag="big")
                        for ci in range(DT):
                            lhs = mixT[:, ci, lj * P:(lj + 1) * P]
                            for dh in range(D // 512):
                                nc.tensor.matmul(
                                    pso[:, dh * 512:(dh + 1) * 512], lhs,
                                    W_outT[:, ci, dh * 512:(dh + 1) * 512],
                                    start=(ci == 0), stop=False,
                                )
                        for ci in range(DT):
                            lhs = qTr[:, ci, lj * P:(lj + 1) * P]
                            for dh in range(D // 512):
                                nc.tensor.matmul(
                                    pso[:, dh * 512:(dh + 1) * 512], lhs,
                                    W_outT[:, DT + ci,
                                           dh * 512:(dh + 1) * 512],
                                    start=False, stop=(ci == DT - 1),
                                )
                        for dh in range(D // 512):
                            o_sb = smp.tile([P, 512], f32, tag="osb", bufs=2)
                            nc.scalar.activation(
                                o_sb[:], pso[:, dh * 512:(dh + 1) * 512],
                                Act.Tanh)
                            nc.gpsimd.dma_start(
                                out_d[b, l0 + lj * P: l0 + (lj + 1) * P,
                                      dh * 512:(dh + 1) * 512],
                                o_sb[:])
                    return qTr_next

                def emit_w_in_setup():
                    # W_in SWDGE cast-loaded to f16, PE-transposed (f16 is
                    # ~2x faster than f32 on the PE weight path).
                    for ei in range(DT):
                        nat = natp.tile([P, D], f16, tag="nat")
                        nc.gpsimd.dma_start(nat[:],
                                            win_d[ei * P:(ei + 1) * P, :])
                        transpose_pack4(
                            nc, W_inT, 0, ei * P,
                            lambda di, nat=nat: nat[:, di * P:(di + 1) * P],
                            DT, ident_bf, f16)

                def emit_w_out_setup():
                    for di in range(DT):
                        for half in range(2):
                            nat = natp.tile([P, D], f16, tag="nat")
                            nc.gpsimd.dma_start(
                                nat[:],
                                wout_d[di * P:(di + 1) * P,
                                       half * D:(half + 1) * D])
                            transpose_pack4(
                                nc, W_outT, 8 * half, di * P,
                                lambda ci, nat=nat: nat[:, ci * P:(ci + 1) * P],
                                DT, ident_bf, f16)

                def emit_all():
                    emit_w_in_setup()
                    emit_ctx_stage(0)
                    qTg = emit_query_stage(0, 0)
                    emit_w_out_setup()
                    qTr = emit_step1(qTg)
                    halves = [(b, h) for b in range(nb) for h in range(NHALF)]
                    for i, (b, h) in enumerate(halves):
                        if h == 0 and b > 0:
                            emit_ctx_stage(b)
                            ctx_tiles.pop(b - 1)
                        nxt = halves[i + 1] if i + 1 < len(halves) else None
                        qTr = emit_half(b, h, qTr, nxt)

                if reps == 1:
                    emit_all()
                else:
                    with tc.For_i(0, reps, 1):
                        emit_all()

    nc.compile()
    return nc


def _get_program(nb, L):
    key = (nb, L)
    if key not in _prog_cache:
        _prog_cache[key] = build_program(nb, L)
    return _prog_cache[key]


def kernel(query, context, mask, W_in, W_out):
    from concourse.bass_utils import run_bass_kernel_spmd

    query = np.ascontiguousarray(query, dtype=np.float32)
    context = np.ascontiguousarray(context, dtype=np.float32)
    W_in = np.ascontiguousarray(W_in, dtype=np.float32)
    W_out = np.ascontiguousarray(W_out, dtype=np.float32)
    B, L, _ = query.shape
    mask3 = np.ascontiguousarray(mask.reshape(B, L, -1), dtype=np.int32)

    nb = B // N_CORES
    nc = _get_program(nb, L)
    in_maps = []
    for c in range(N_CORES):
        b0 = c * nb
        in_maps.append({
            "query": query[b0:b0 + nb],
            "context": context[b0:b0 + nb],
            "mask": mask3[b0:b0 + nb],
            "W_in": W_in,
            "W_out": W_out,
        })
    res = run_bass_kernel_spmd(nc, in_maps, core_ids=list(range(N_CORES)))
    global last_results
    last_results = res
    out = np.concatenate([r["out"] for r in res.results], axis=0)
    return out
